# revision 39
# baseline (speedup 1.0000x reference)
"""GCN discriminator kernel for 8 Trainium2 NeuronCores.

Strategy (edge-parallel by destination, V3): all index-derived math is
done on host: degrees, s = 1/sqrt(deg).  The symmetric edge norm
s[src]*s[dst] is split: s[src] is folded into the gathered table
(xb[i] = s[i]*x[i], fp8 e4m3 — halves the random-gather HBM bytes; the
PE accepts the mixed bf16-lhsT x fp8-rhs matmul) and s[dst] is applied
per-partition in the block epilogue (activation Copy with a
per-partition scale AP), so the one-hot is pure 0/1 and needs a single
DVE is_equal per chunk.

Real edges are bucketed by (core = dst // 6250, dst block of 128,
src lo/hi int16 segment) and padded to whole 128-edge tiles (pad:
idx=0, col=128).  Self-loops are NOT gathered: their rows are block-
contiguous, so a single sequential DMA stages each core's own dst-
shard rows (xself, fp8) and a per-block diag(s_dst) matmul adds the
self term into PSUM before the accumulation closes — this removes
~93 padded tiles (~10%) from the random-gather stream.  Each core
runs one pass over its ~848 tiles:
  - dma_gather pulls up to CHUNK=8 tiles (the HW ucode limit is 1024
    indices per call) of fp8 prescaled source rows per call, spanning
    dst blocks within a same-segment run.  The idx table is split into
    a small head tile + tail tile so the first gather starts early.
  - per chunk, ONE broadcast-free DVE is_equal builds the one-hot
    oh[p, j, t] = (cl[p, t] == j) in bf16 against an on-chip iota
    table io2[p, j, t] = j; the [p, j, t] layout keeps the last
    axis packed for all operands, enabling the DVE 2x mode (the old
    [p, t, j] layout put a stride-0 broadcast on the last axis, which
    forced 1x and made DVE the bottleneck).
  - per tile, one bf16 matmul accumulates oh[:, :, t]^T @ xrows into
    the dst block's PSUM [128,256].
  - per dst block epilogue: diag(s_dst) @ x_block self-loop matmul
    closes the PSUM group; then y -> bf16 with per-partition s[dst] scale,
    transpose, z = y@W + b_conv, sigmoid, and a ones-vector matmul
    accumulates the column sum of sigmoid rows for the mean.
Blocks are processed in groups of 4 (lo tiles of the group, then hi
tiles) so gather calls span blocks while only ~8 PSUM banks are live.
Each core emits only its scalar partial dot (w_lin/N) . sum(sigmoid);
the cross-core sum, b_lin add, and final sigmoid run on host, which
removes a ~15-28us collective tail from the device timeline.
"""

import sys

for _p in ("/opt/trn_rl_repo", "/root/.axon_site/_ro/trn_rl_repo"):
    if _p not in sys.path:
        sys.path.insert(0, _p)

import numpy as np

N = 50000
E = 800000
D = 256
C = 8            # cores
NS = N // C      # dst rows per core
P = 128
B = (NS + P - 1) // P          # dst blocks per core (49; last has 106 rows)
LAST_ROWS = NS - (B - 1) * P   # 106
SPLIT = 32768                  # int16 index reach of dma_gather
CHUNK = 8                      # max tiles per dma_gather call
G = 4                          # dst blocks per group (bounds live PSUM tiles)
IDXW = P // 16                 # idx columns per tile (wrapped in 16 parts)

_cache = {}


def _schedule(T_seg):
    """Static tile order shared by host prep and program build.

    Blocks are grouped G at a time; within a group all lo-segment tiles
    come first (chunked runs spanning blocks), then all hi tiles.
    Returns (tiles, first, last, chunks, tile_index) where
    tiles[j] = (block, seg), chunks = (j0, ntiles, seg), and
    tile_index[(b, seg)] = first global tile index of that run.
    """
    tiles = []
    tile_index = {}
    for g0 in range(0, B, G):
        for seg in range(2):
            for b in range(g0, min(g0 + G, B)):
                tile_index[(b, seg)] = len(tiles)
                tiles += [(b, seg)] * T_seg[b][seg]
    chunks = []
    j = 0
    T = len(tiles)
    while j < T:
        seg = tiles[j][1]
        nt = 1
        while nt < CHUNK and j + nt < T and tiles[j + nt][1] == seg:
            nt += 1
        chunks.append((j, nt, seg))
        j += nt
    first = {}
    last = {}
    for jj, (b, _) in enumerate(tiles):
        first.setdefault(b, jj)
        last[b] = jj
    return tiles, first, last, chunks, tile_index


def _prep_host(edge_index):
    """Host-side index math + bucketing.

    Computes deg/s, buckets edges (+self loops) by (core, dst block,
    lo/hi source segment), pads every (block,seg) run to the max
    whole-tile count across cores.  Pad slots: idx=0, col=128 (one-hot
    row all-zero).  Returns per-core idx/cl tables, the per-core dst
    scale table scol [128, B], the global node scale s (for prescaling
    x), and the schedule shape.
    """
    import ml_dtypes

    r_all = np.asarray(edge_index[0], np.int64)
    c_all = np.asarray(edge_index[1], np.int64)
    # self-loops are NOT bucketed as edges; they are applied on-device by a
    # per-block diag(s) matmul over a sequentially-loaded block-row table.
    deg = (np.bincount(c_all, minlength=N) + 1).astype(np.float64)
    s = 1.0 / np.sqrt(deg)                      # every node has a self-loop

    core = c_all // NS
    rem = c_all % NS
    cl = (rem % P).astype(np.float32)
    seg = (r_all >= SPLIT).astype(np.int64)
    bucket = (core * B + rem // P) * 2 + seg
    order = np.argsort(bucket, kind="stable")
    counts = np.bincount(bucket, minlength=C * B * 2).reshape(C, B, 2)
    T_seg = (-(-counts // P)).max(axis=0)       # [B, 2]
    # every block needs >= 1 tile so its epilogue (self-loop matmul, mean
    # accumulation) is emitted even if it has no real in-edges
    T_seg[:, 0] = np.maximum(T_seg[:, 0], 1)

    tiles, first, last, chunks, tile_index = _schedule(
        [[int(v) for v in x] for x in T_seg])
    T = len(tiles)

    # trailing-pad trim: a chunk whose last tile closes a (block, seg)
    # bucket can statically skip that bucket's pad tail (the slots no
    # core fills).  The first 16 chunks stay untrimmed so every xg pool
    # buffer is fully written once before any trimmed call can leave
    # stale trailing slots (zero one-hot columns make them harmless,
    # but only if they hold valid fp8 values, never uninitialized).
    maxcnt = counts.max(axis=0)                 # [B, 2]
    trims = []
    for ii, (j0, nt, sg) in enumerate(chunks):
        b, _ = tiles[j0 + nt - 1]
        trim = 0
        if ii >= 16 and j0 + nt == tile_index[(b, sg)] + int(T_seg[b][sg]):
            # cap below P so num_idxs stays >= 1 even for a forced
            # pad-only tile (empty bucket)
            trim = min(int(T_seg[b][sg]) * P - int(maxcnt[b, sg]), P - 1)
            assert 0 <= trim
        trims.append(trim)

    starts = np.zeros(C * B * 2 + 1, np.int64)
    np.cumsum(counts.reshape(-1), out=starts[1:])

    r_t = np.zeros((C, T * P), np.int64)            # pad idx -> row 0
    cl_p = np.full((C, T * P), P, np.float32)       # pad col -> 128
    rs = r_all[order]
    cs = cl[order]
    for ci in range(C):
        for b in range(B):
            for sg in range(2):
                cnt = int(counts[ci, b, sg])
                s0 = int(starts[(ci * B + b) * 2 + sg])
                d0 = tile_index[(b, sg)] * P
                r_t[ci, d0:d0 + cnt] = rs[s0:s0 + cnt] - (SPLIT if sg else 0)
                cl_p[ci, d0:d0 + cnt] = cs[s0:s0 + cnt]

    # [C, T*P] -> [C, P, T]: tile j is column j, edge slot q is partition q
    bff = ml_dtypes.bfloat16
    cl_p = np.ascontiguousarray(
        cl_p.reshape(C, T, P).transpose(0, 2, 1)).astype(bff)
    # idx param: element i of a call at [i % 16, col0 + i // 16],
    # replicated 8x down the partitions.  Calls are tile-aligned, so
    # tile j owns idx columns [j*IDXW, (j+1)*IDXW).
    idx16 = r_t.reshape(C, T * IDXW, 16).transpose(0, 2, 1).astype(np.int16)
    idx_p = np.ascontiguousarray(np.tile(idx16, (1, 8, 1)))  # [C, 128, T*8]

    # per-core dst scale: scol[r, b] = s[core*NS + b*128 + r] (f32)
    scol = np.zeros((C, P, B), np.float32)
    sv = s.astype(np.float32)
    for ci in range(C):
        block = sv[ci * NS:(ci + 1) * NS]
        pad = np.zeros(B * P - NS, np.float32)
        scol[ci] = np.concatenate([block, pad]).reshape(B, P).T

    return (idx_p, cl_p, scol, sv, [[int(v) for v in x] for x in T_seg],
            T, trims)


def _build(T_seg, T, trims):
    from concourse import bass, bacc, mybir
    import concourse.tile as tile
    from concourse.masks import make_identity

    f32 = mybir.dt.float32
    bf16 = mybir.dt.bfloat16
    i16 = mybir.dt.int16

    nc = bacc.Bacc(
        "TRN2",
        target_bir_lowering=False,
        debug=False,
        num_devices=C,
        num_swdge_queues=1,
        dynamic_dma_scratch_size=16384,
    )

    f8 = mybir.dt.float8e4
    xb_d = nc.declare_dram_parameter("xb", [N, D], f8, isOutput=False)
    xs_d = nc.declare_dram_parameter("xself", [P, B, D], f8, isOutput=False)
    idx_d = nc.declare_dram_parameter("idx", [P, T * IDXW], i16, isOutput=False)
    cl_d = nc.declare_dram_parameter("cl", [P, T], bf16, isOutput=False)
    sc_d = nc.declare_dram_parameter("scol", [P, B], f32, isOutput=False)
    W_d = nc.declare_dram_parameter("W", [D, D], bf16, isOutput=False)
    bc_d = nc.declare_dram_parameter("bconv", [1, D], bf16, isOutput=False)
    wl_d = nc.declare_dram_parameter("wlin", [1, D], f32, isOutput=False)
    out_d = nc.declare_dram_parameter("out", [1, 1], f32, isOutput=True)

    tiles, first, last, chunks, tile_index = _schedule(T_seg)
    # idx is split into two tiles so the first gather only waits on the
    # small head portion (first two block groups), not the full ~1.9 MB.
    K = tile_index[(2 * G, 0)] if B > 2 * G else T

    with tile.TileContext(nc) as tc:
        with tc.tile_pool(name="static", bufs=1) as st, \
             tc.tile_pool(name="oh", bufs=6) as ohp, \
             tc.tile_pool(name="xg", bufs=16) as xgp, \
             tc.tile_pool(name="ep", bufs=4) as epp, \
             tc.tile_pool(name="py", bufs=G + 1, space="PSUM") as pyp, \
             tc.tile_pool(name="pt", bufs=1, space="PSUM") as ptp, \
             tc.tile_pool(name="pz", bufs=1, space="PSUM") as pzp, \
             tc.tile_pool(name="pm", bufs=1, space="PSUM") as pmp:

            # ---- static loads ----
            idxA_sb = st.tile([P, K * IDXW], i16, tag="idxA")
            idxB_sb = st.tile([P, (T - K) * IDXW], i16, tag="idxB")
            cl_sb = st.tile([P, T], bf16, tag="cl")
            nc.sync.dma_start(out=idxA_sb[:], in_=idx_d[:, 0:K * IDXW])
            nc.sync.dma_start(out=idxB_sb[:], in_=idx_d[:, K * IDXW:])
            nc.sync.dma_start(out=cl_sb[:], in_=cl_d[:])
            sc_sb = st.tile([P, B], f32, tag="scol")
            nc.sync.dma_start(out=sc_sb[:], in_=sc_d[:])
            xself_sb = st.tile([P, B, D], f8, tag="xself")
            nc.sync.dma_start(out=xself_sb[:], in_=xs_d[:])
            # io2[p, j, t] = j, built on-chip (saves static DMA traffic)
            io2_sb = st.tile([P, P, CHUNK], bf16, tag="io2")
            nc.gpsimd.iota(
                io2_sb[:], pattern=[[1, P], [0, CHUNK]],
                channel_multiplier=0,
                allow_small_or_imprecise_dtypes=True,
            )
            W0_sb = st.tile([P, D], bf16, tag="w0")
            W1_sb = st.tile([P, D], bf16, tag="w1")
            nc.sync.dma_start(out=W0_sb[:], in_=W_d[0:P, :])
            nc.sync.dma_start(out=W1_sb[:], in_=W_d[P:D, :])
            bc_sb = st.tile([1, D], bf16, tag="bc")
            nc.sync.dma_start(out=bc_sb[:], in_=bc_d[:])
            wl_sb = st.tile([1, D], f32, tag="wl")
            nc.sync.dma_start(out=wl_sb[:], in_=wl_d[:])
            ident = st.tile([P, P], bf16, tag="ident")
            make_identity(nc, ident[:])
            ones_c = st.tile([P, 1], bf16, tag="onesc")
            nc.vector.memset(ones_c[:], 1.0)
            ones_r = st.tile([1, P], bf16, tag="onesr")
            nc.vector.memset(ones_r[:], 1.0)

            mean_ps = pmp.tile([1, D], f32, tag="mean")
            xb_lo = xb_d[0:SPLIT, :]
            xb_hi = xb_d[SPLIT:N, :]
            y_of = dict(first)          # block -> (first tile j, psum tile)

            for ci, (j0, nt, seg) in enumerate(chunks):
                xg = xgp.tile([P, CHUNK, D], f8, tag="xg")
                idxs = (idxA_sb[:, j0 * IDXW:(j0 + nt) * IDXW]
                        if j0 < K else
                        idxB_sb[:, (j0 - K) * IDXW:(j0 - K + nt) * IDXW])
                nid = nt * P - trims[ci]
                nc.gpsimd.dma_gather(
                    xg[:, 0:nt, :],
                    xb_lo if seg == 0 else xb_hi,
                    idxs,
                    nid,
                    nid,
                    D,
                    queue_num=0,
                )
                # one-hot oh[p, j, t] = (cl[p, j0+t] == j); all operands
                # keep a packed last axis so the DVE 2x mode applies.
                oh = ohp.tile([P, P, CHUNK], bf16, tag="oh")
                nc.vector.tensor_tensor(
                    out=oh[:, :, 0:nt],
                    in0=cl_sb[:, None, j0:j0 + nt].to_broadcast((P, P, nt)),
                    in1=io2_sb[:, :, 0:nt],
                    op=mybir.AluOpType.is_equal,
                )
                for kk in range(nt):
                    j = j0 + kk
                    b = tiles[j][0]
                    if j == y_of[b]:
                        y_ps = pyp.tile([P, D], f32, tag="y")
                        y_of[b] = (j, y_ps)
                    else:
                        y_ps = y_of[b][1]
                    nc.tensor.matmul(
                        out=y_ps[:],
                        lhsT=oh[:, :, kk],
                        rhs=xg[:, kk, :],
                        start=(y_of[b][0] == j),
                        stop=False,
                    )
                    if last[b] != j:
                        continue
                    # ---- block epilogue ----
                    # self-loop term: y += diag(s_dst) @ x_block (epilogue
                    # scol scale then makes it s^2 x); block rows come from
                    # the sequentially-loaded xself table, not the gather.
                    diag = epp.tile([P, P], bf16, tag="diag")
                    nc.vector.tensor_scalar_mul(
                        diag[:], ident[:], sc_sb[:, b:b + 1])
                    nc.tensor.matmul(
                        out=y_ps[:],
                        lhsT=diag[:],
                        rhs=xself_sb[:, b, :],
                        start=False,
                        stop=True,
                    )
                    y_sb = epp.tile([P, D], bf16, tag="ysb")
                    nc.scalar.activation(
                        y_sb[:], y_ps[:], mybir.ActivationFunctionType.Copy,
                        scale=sc_sb[:, b:b + 1],
                    )
                    z_ps = pzp.tile([P, D], f32, tag="z")
                    for h in range(2):
                        yt_ps = ptp.tile([P, P], bf16, tag="yt")
                        nc.tensor.transpose(
                            out=yt_ps[:],
                            in_=y_sb[:, h * P:(h + 1) * P],
                            identity=ident[:],
                        )
                        yt_sb = epp.tile([P, P], bf16, tag="ytsb")
                        nc.vector.tensor_copy(out=yt_sb[:], in_=yt_ps[:])
                        nc.tensor.matmul(
                            out=z_ps[:],
                            lhsT=yt_sb[:],
                            rhs=(W0_sb if h == 0 else W1_sb)[:],
                            start=(h == 0),
                            stop=False,
                        )
                    nc.tensor.matmul(
                        out=z_ps[:],
                        lhsT=ones_r[:],
                        rhs=bc_sb[:],
                        start=False,
                        stop=True,
                    )
                    sig = epp.tile([P, D], bf16, tag="sig")
                    nc.scalar.activation(
                        sig[:], z_ps[:], mybir.ActivationFunctionType.Sigmoid,
                    )
                    rows = LAST_ROWS if b == B - 1 else P
                    nc.tensor.matmul(
                        out=mean_ps[:],
                        lhsT=ones_c[0:rows, :],
                        rhs=sig[0:rows, :],
                        start=(b == 0),
                        stop=(b == B - 1),
                    )

            # ---- final: local dot with w_lin/N; the cross-core sum, b_lin
            # add, and sigmoid happen on host (w.(sum_c m_c)/N + b =
            # sum_c ((w/N).m_c) + b), avoiding a ~15us collective tail.
            prod = epp.tile([1, D], f32, tag="prod")
            nc.vector.tensor_tensor(
                out=prod[:], in0=mean_ps[:], in1=wl_sb[:], op=mybir.AluOpType.mult,
            )
            dot = epp.tile([1, 1], f32, tag="dot")
            nc.vector.tensor_reduce(
                out=dot[:], in_=prod[:], axis=mybir.AxisListType.X,
                op=mybir.AluOpType.add,
            )
            nc.sync.dma_start(out=out_d[:], in_=dot[:])

    nc.compile()
    return nc


def _make_in_maps(inputs, idx_p, cl_p, scol, sv):
    import ml_dtypes

    bf16 = ml_dtypes.bfloat16
    xs = (np.asarray(inputs["x"], np.float32) * sv[:, None]).astype(
        ml_dtypes.float8_e4m3)
    xsp = np.zeros((C, B * P, D), ml_dtypes.float8_e4m3)
    for ci in range(C):
        xsp[ci, 0:NS] = xs[ci * NS:(ci + 1) * NS]
    # [B*P, D] -> [P, B, D]: partition = row-within-block
    xsp = np.ascontiguousarray(
        xsp.reshape(C, B, P, D).transpose(0, 2, 1, 3))
    common = {
        "xb": np.ascontiguousarray(xs),
        "W": np.asarray(inputs["W"], np.float32).astype(bf16),
        "bconv": np.asarray(inputs["b_conv"], np.float32).reshape(1, D).astype(bf16),
        "wlin": np.asarray(inputs["w_lin"], np.float32).reshape(1, D) / N,
    }
    return [
        {**common, "idx": idx_p[ci], "cl": cl_p[ci], "scol": scol[ci],
         "xself": xsp[ci]}
        for ci in range(C)
    ]


def kernel(x, edge_index, W, b_conv, w_lin, b_lin):
    from concourse.bass_utils import run_bass_kernel_spmd

    idx_p, cl_p, scol, sv, T_seg, T, trims = _prep_host(edge_index)

    key = (tuple(tuple(t) for t in T_seg), tuple(trims))
    if key not in _cache:
        _cache.clear()
        _cache[key] = _build(T_seg, T, trims)
    nc = _cache[key]

    in_maps = _make_in_maps(
        {"x": x, "W": W, "b_conv": b_conv, "w_lin": w_lin, "b_lin": b_lin},
        idx_p, cl_p, scol, sv,
    )
    res = run_bass_kernel_spmd(nc, in_maps, list(range(C)))
    dsum = np.float64(0.0)
    for ci in range(C):
        dsum += np.float64(res.results[ci]["out"].reshape(()))
    z = dsum + np.float64(np.asarray(b_lin, np.float32).reshape(()))
    out = 1.0 / (1.0 + np.exp(-z))
    return np.asarray([out], dtype=np.float32)
